# revision 2
# baseline (speedup 1.0000x reference)
"""Trainium2 Bass kernel for the STFT patch-dispatch loss.

Same hop-folded DFT as the baseline, engineered around engine balance:

  - fp16 everywhere on the signal path (11-bit mantissa = f32r-grade
    precision at half the bytes): V, folds, DFT weights/operands, mags,
    patch diffs.  All big DVE ops run in 16-bit 2x mode.
  - vpp / vq computed with tensor_tensor_scan sliding-window recurrences
    straight from V (no vp intermediate).
  - Each DFT range lands re|im packed in one 2-bank PSUM tile -> one ACT
    Square per range + one fp16 DVE add -> |X|^2, then one ACT Sqrt per
    signal.
  - t in [0,1024) everywhere on device; the tail frame (t=1024) and the
    nyquist row (k=512) are shipped raw and folded in on the host.
  - Waveform loads consolidated into 5 DMAs per signal-row.

Sharding: batch rows 2c, 2c+1 -> core c (8 cores).
"""
import numpy as np

import concourse.bass as bass
import concourse.bacc as bacc
import concourse.mybir as mybir
from concourse import tile

dt = mybir.dt
Alu = mybir.AluOpType
Act = mybir.ActivationFunctionType

B, L = 16, 262144
NCORES = 8
RPC = B // NCORES
NFFT, HOP, PS = 1024, 256, 16
PAD = NFFT // 2
LP = L + 2 * PAD           # 263168
NBLK = LP // HOP           # 1028
T = 1 + (LP - NFFT) // HOP  # 1025 frames; main block = 1024, tail = 1
TM = T - 1                 # 1024
NPF, NPT = 33, 65
KSEL = max(1, int(NPF * NPT * 0.3))
EPS = 1e-08

WDT = dt.float16           # working dtype on the signal path
MAIN_RANGES = [(0, 512), (512, 1024)]


def _consts():
    r = np.arange(256)
    p = np.arange(128)
    wc = np.empty((256, 512), np.float32)
    ws = np.empty((256, 512), np.float32)
    for c in range(4):
        k = 4 * p + c
        ang = 2.0 * np.pi * np.outer(r, k) / NFFT
        wc[:, 128 * c:128 * (c + 1)] = np.cos(ang)
        ws[:, 128 * c:128 * (c + 1)] = -np.sin(ang)
    wc = wc.astype(np.float16)
    ws = ws.astype(np.float16)
    wcn = np.concatenate([-wc[:, 128:256], -wc[:, 384:512],
                          -wc[:, 256:384]], axis=1)
    wsn = np.concatenate([-ws[:, 128:256], -ws[:, 384:512],
                          -ws[:, 256:384]], axis=1)
    wn = np.where(r % 2 == 0, 1.0, -1.0).astype(np.float16).reshape(256, 1)
    ones4 = (p[:, None] // 4 == np.arange(32)[None, :]).astype(np.float32)
    ident = np.eye(128, dtype=np.float32)
    swap2 = np.array([[0.0, 1.0], [1.0, 0.0]], np.float32)
    return {
        "wc0": wc[:128], "wc1": wc[128:],
        "ws0": ws[:128], "ws1": ws[128:],
        "wn0": wn[:128], "wn1": wn[128:],
        "wcn0": wcn[:128], "wcn1": wcn[128:],
        "wsn0": wsn[:128], "wsn1": wsn[128:],
        "ones4": ones4, "ident": ident, "swap2": swap2,
    }


CONST_SPECS = {
    "wc0": [128, 512], "wc1": [128, 512],
    "ws0": [128, 512], "ws1": [128, 512],
    "wn0": [128, 1], "wn1": [128, 1],
    "wcn0": [128, 384], "wcn1": [128, 384],
    "wsn0": [128, 384], "wsn1": [128, 384],
    "ones4": [128, 32], "ident": [128, 128], "swap2": [2, 2],
}
FP16_CONSTS = ("wc0", "wc1", "ws0", "ws1", "wn0", "wn1",
               "wcn0", "wcn1", "wsn0", "wsn1")


def _seg(x_d, b, start, nrows):
    return x_d[b:b + 1, start:start + 256 * nrows].rearrange(
        "o (m r) -> (o m) r", r=256)


def build_nc(repeat=1):
    nc = bacc.Bacc("TRN2", target_bir_lowering=False, debug=False,
                   num_devices=NCORES)

    x_d = {s: nc.dram_tensor(f"x{s}", [RPC, L], dt.float32,
                             kind="ExternalInput") for s in "stg"}
    c_d = {n: nc.dram_tensor(
        n, shp, dt.float16 if n in FP16_CONSTS else dt.float32,
        kind="ExternalInput") for n, shp in CONST_SPECS.items()}
    osum_d = nc.dram_tensor("osum", [RPC * 3, 32, 64], dt.float32,
                            kind="ExternalOutput")
    nyq_d = nc.dram_tensor("nyq", [RPC * 3, T], dt.float32,
                           kind="ExternalOutput")
    tail_d = nc.dram_tensor("tail", [RPC * 3, 128, 8], dt.float32,
                            kind="ExternalOutput")

    with tile.TileContext(nc) as tc:
        with (
            tc.tile_pool(name="const", bufs=1) as cp,
            tc.tile_pool(name="vwork", bufs=2) as vp_,
            tc.tile_pool(name="fwork", bufs=2) as fp_,
            tc.tile_pool(name="upool", bufs=3) as up,
            tc.tile_pool(name="magpool", bufs=2) as magp,
            tc.tile_pool(name="sqpool", bufs=3) as sqp,
            tc.tile_pool(name="dpool", bufs=3) as dp_,
            tc.tile_pool(name="redpool", bufs=3) as redp,
            tc.tile_pool(name="dft_ps", bufs=2, space="PSUM") as dft_ps,
            tc.tile_pool(name="tr_ps", bufs=1, space="PSUM") as tr_ps,
            tc.tile_pool(name="pa_ps", bufs=2, space="PSUM") as pa_ps,
            tc.tile_pool(name="ny_ps", bufs=1, space="PSUM") as ny_ps,
        ):
            C = {}
            for n, shp in CONST_SPECS.items():
                cdt = dt.float16 if n in FP16_CONSTS else dt.float32
                C[n] = cp.tile(shp, cdt, tag=n, name=f"c_{n}")
                nc.sync.dma_start(C[n][:], c_d[n][:])

            def rev2(seg_start, s, b):
                """[2,256]: rows = rev(seg rows) in place; the consumer
                transposes with `swap2` so the row order comes out right."""
                sc = up.tile([2, 256], dt.float32, tag="sc", name="sc")
                nc.sync.dma_start(sc[:], _seg(x_d[s], b, seg_start, 2))
                ur = up.tile([2, 256], dt.float32, tag="ur", name="ur")
                nc.vector.tensor_copy(ur[:], sc[0:2, 255::-1])
                return ur

            def build_V(s, b):
                """V[r, h, m]: [128, 2, NBLK] fp16 tile."""
                v = vp_.tile([128, 2, NBLK], WDT, tag="v")
                uh = rev2(1, s, b)
                u0 = up.tile([128, 256], dt.float32, tag="u0", name="u0")
                nc.sync.dma_start(u0[0:126, :], _seg(x_d[s], b, 0, 126))
                um = up.tile([128, 7, 256], dt.float32, tag="um", name="um")
                nc.sync.dma_start(
                    um[:], x_d[s][b:b + 1, 32256:32256 + 7 * 32768].rearrange(
                        "o (g m r) -> (o m) g r", m=128, r=256))
                ut = up.tile([2, 256], dt.float32, tag="ut", name="ut")
                nc.sync.dma_start(ut[:], _seg(x_d[s], b, 261632, 2))
                ub = rev2(261631, s, b)
                groups = [
                    (0, [(uh[0:2, :], 2, True), (u0[0:126, :], 126, False),
                         (um[:, 0, :], 128, False), (um[:, 1, :], 128, False),
                         (um[:, 2, :], 128, False)]),
                    (512, [(um[:, 3, :], 128, False), (um[:, 4, :], 128, False),
                           (um[:, 5, :], 128, False),
                           (um[:, 6, :], 128, False)]),
                    (1024, [(ut[0:2, :], 2, False), (ub[0:2, :], 2, True)]),
                ]
                for col, pieces in groups:
                    width = sum(nr for _, nr, _ in pieces)
                    for h in (0, 1):
                        tp = tr_ps.tile([128, 512], dt.float32, tag="trp",
                                        name="tp")
                        off = 0
                        for uap, nr, swp in pieces:
                            rhs = (C["swap2"][0:2, 0:2] if swp
                                   else C["ident"][0:nr, 0:nr])
                            nc.tensor.transpose(
                                tp[:, off:off + nr],
                                uap[:, 128 * h:128 * h + 128], rhs)
                            off += nr
                        nc.scalar.copy(v[:, h, col:col + width],
                                       tp[:, 0:width])
                return v

            def fold_ops(v):
                """vm [128,2,1026] (TT); vpp, vq [128,2,1025] (scans)."""
                vm = fp_.tile([128, 2, NBLK - 2], WDT, tag="vm")
                nc.vector.tensor_sub(vm[:], v[:, :, 0:NBLK - 2],
                                     v[:, :, 2:NBLK])
                vpp = fp_.tile([128, 2, T], WDT, tag="vpp")
                vq = fp_.tile([128, 2, T], WDT, tag="vq")
                a2 = fp_.tile([128, 2, 2], WDT, tag="a2")
                nc.vector.tensor_add(a2[:], v[:, :, 0:2], v[:, :, 2:4])
                nc.vector.tensor_add(vpp[:, :, 0:1], a2[:, :, 0:1],
                                     a2[:, :, 1:2])
                nc.vector.tensor_sub(vq[:, :, 0:1], a2[:, :, 0:1],
                                     a2[:, :, 1:2])
                for h in (0, 1):
                    # vpp[t] = (v[t+3] + vpp[t-1]) - v[t-1]
                    nc.vector.tensor_tensor_scan(
                        vpp[:, h, 1:T], v[:, h, 4:3 + T], v[:, h, 0:T - 1],
                        initial=vpp[:, h, 0:1], op0=Alu.add,
                        op1=Alu.subtract)
                    # vq[t] = (v[t-1] - vq[t-1]) - v[t+3]
                    nc.vector.tensor_tensor_scan(
                        vq[:, h, 1:T], v[:, h, 0:T - 1], v[:, h, 4:3 + T],
                        initial=vq[:, h, 0:1], op0=Alu.subtract,
                        op1=Alu.subtract)
                return vm, vpp, vq

            def chunk_terms(c):
                cs = slice(128 * c, 128 * (c + 1))
                if c % 2 == 0:
                    return ([("wc", cs, 0)], [("ws", cs, 0)])
                ns = slice(0, 128) if c == 1 else slice(128, 256)
                if c == 1:
                    return ([("wc", cs, 0), ("ws", cs, 1)],
                            [("ws", cs, 0), ("wcn", ns, 1)])
                return ([("wc", cs, 0), ("wsn", ns, 1)],
                        [("ws", cs, 0), ("wc", cs, 1)])

            def chunk_mag(ops, c, mag2, score, sig_idx):
                """DFT chunk c -> mag2[:, c*1024 + t] (fp16) + tail cols."""
                vm, vpp, vq = ops
                terms_re, terms_im = chunk_terms(c)
                src = (vpp if c == 0 else vq) if c % 2 == 0 else vm
                for ri, (lo, hi) in enumerate(MAIN_RANGES):
                    ps = dft_ps.tile([128, 1024], dt.float32, tag="dftp",
                                     name="psx")
                    for xi, terms in enumerate((terms_re, terms_im)):
                        nmm = 2 * len(terms)
                        k = 0
                        for wname, wsl, shift in terms:
                            for h in (0, 1):
                                nc.tensor.matmul(
                                    ps[:, 512 * xi:512 * xi + 512],
                                    C[wname + str(h)][:, wsl],
                                    src[:, h, lo + shift:hi + shift],
                                    start=(k == 0), stop=(k == nmm - 1))
                                k += 1
                    sq = sqp.tile([128, 1024], WDT, tag="sqt", name="sq")
                    nc.scalar.activation(sq[:], ps[:], Act.Square)
                    nc.vector.tensor_add(
                        mag2[:, c * TM + lo:c * TM + hi],
                        sq[:, 0:512], sq[:, 512:1024])
                # tail t=1021..1024 (4-wide; small-N MM limits) ->
                # score cols 192 + 32*sig + 8*c + 4*xi
                base = 192 + 32 * sig_idx + 8 * c
                for xi, terms in enumerate((terms_re, terms_im)):
                    nmm = 2 * len(terms)
                    k = 0
                    for wname, wsl, shift in terms:
                        for h in (0, 1):
                            nc.tensor.matmul(
                                score[:, base + 4 * xi:base + 4 * xi + 4],
                                C[wname + str(h)][:, wsl],
                                src[:, h, TM - 3 + shift:TM + 1 + shift],
                                start=(k == 0), stop=(k == nmm - 1))
                            k += 1

            def nyq_rows(ops, idx):
                """X[512, :] (real) -> nyq_d[idx] (PSUM -> SBUF -> DRAM)."""
                _, vpp, _ = ops
                nsb = redp.tile([1, T], dt.float32, tag="nyqsb", name="nsb")
                for lo, hi in [(0, 512), (512, 1024), (1021, 1025)]:
                    pn = ny_ps.tile([1, 512], dt.float32, tag="nyp",
                                    name="pn")
                    nc.tensor.matmul(pn[:, 0:hi - lo], C["wn0"][:, 0:1],
                                     vpp[:, 0, lo:hi], start=True, stop=False)
                    nc.tensor.matmul(pn[:, 0:hi - lo], C["wn1"][:, 0:1],
                                     vpp[:, 1, lo:hi], start=False, stop=True)
                    nc.scalar.copy(nsb[:, lo:hi], pn[:, 0:hi - lo])
                nc.sync.dma_start(nyq_d[idx:idx + 1, :], nsb[:])

            def signal_mags(ops, sig_idx, b, score):
                """All 4 chunks -> mag [128, 4096] fp16 (+ tail, nyq out)."""
                mag2 = magp.tile([128, 4 * TM], WDT, tag="mag2",
                                 name="mag2")
                for c in range(4):
                    chunk_mag(ops, c, mag2, score, sig_idx)
                idx = b * 3 + sig_idx
                tbase = 192 + 32 * sig_idx
                tsb = redp.tile([128, 8], dt.float32, tag="tailsb",
                                name="tsb")
                nc.vector.tensor_copy(
                    tsb[:], score[:, tbase + 3:tbase + 32:4])
                nc.sync.dma_start(
                    tail_d[idx:idx + 1].rearrange("o p f -> (o p) f"),
                    tsb[:])
                nyq_rows(ops, idx)
                mag = magp.tile([128, 4 * TM], WDT, tag=f"mag{sig_idx}",
                                name=f"mag{sig_idx}")
                nc.scalar.activation(mag[:], mag2[:], Act.Sqrt)
                return mag

            def patch(b, ms, mt, mg, score):
                for mi, (ta, tb, sqr) in enumerate(
                        ((ms, mg, False), (mt, mg, False), (ms, mt, True))):
                    d = dp_.tile([128, 4 * TM], WDT, tag="d", name=f"d{mi}")
                    nc.vector.tensor_sub(d[:], ta[:], tb[:])
                    if sqr:
                        d2 = dp_.tile([128, 4 * TM], WDT, tag="d",
                                      name="d2")
                        nc.vector.tensor_mul(d2[:], d[:], d[:])
                        d = d2
                    red = redp.tile([128, 256], dt.float32, tag="red",
                                    name=f"red{mi}")
                    nc.vector.tensor_reduce(
                        red[:],
                        d[:].rearrange("p (a t) -> p a t", t=16),
                        axis=mybir.AxisListType.X, op=Alu.add,
                        apply_absolute_value=not sqr)
                    pps = score[0:32, 64 * mi:64 * mi + 64]
                    for c in range(4):
                        nc.tensor.matmul(pps, C["ones4"][:],
                                         red[:, 64 * c:64 * (c + 1)],
                                         start=(c == 0), stop=(c == 3))
                    idx = b * 3 + mi
                    osb = redp.tile([32, 64], dt.float32, tag="osb",
                                    name="osb")
                    nc.vector.tensor_copy(osb[:], pps)
                    nc.sync.dma_start(
                        osum_d[idx:idx + 1].rearrange("o p f -> (o p) f"),
                        osb[:])

            def row_stage_a(b):
                score = pa_ps.tile([128, 288], dt.float32, tag="score",
                                   name="score")
                vs = build_V("s", b)
                ops = fold_ops(vs)
                vt = build_V("t", b)
                ms = signal_mags(ops, 0, b, score)
                ops = fold_ops(vt)
                vg = build_V("g", b)
                mt_ = signal_mags(ops, 1, b, score)
                ops = fold_ops(vg)
                mg = signal_mags(ops, 2, b, score)
                return (b, ms, mt_, mg, score)

            def body():
                for b in range(RPC):
                    patch(*row_stage_a(b))

            if repeat == 1:
                body()
            else:
                with tc.For_i(0, repeat, 1):
                    body()

    nc.compile()
    return nc


_NC_CACHE = {}


def _get_nc():
    if "nc" not in _NC_CACHE:
        _NC_CACHE["nc"] = build_nc()
    return _NC_CACHE["nc"]


def _run_on_cores(nc, in_maps):
    """Execute via cached PJRT callable (axon) with jit reuse."""
    from concourse.bass_utils import axon_active

    if not axon_active():
        from concourse.bass_utils import run_bass_kernel_spmd
        return run_bass_kernel_spmd(nc, in_maps,
                                    core_ids=list(range(NCORES))).results

    import jax
    from jax.sharding import Mesh, PartitionSpec
    from jax.experimental.shard_map import shard_map
    from concourse import bass2jax

    key = id(nc)
    if key not in _NC_CACHE.setdefault("jit", {}):
        bass2jax.install_neuronx_cc_hook()
        part_name = (nc.partition_id_tensor.name
                     if nc.partition_id_tensor else None)
        in_names, out_names, out_avals, zero_outs = [], [], [], []
        for alloc in nc.m.functions[0].allocations:
            if not isinstance(alloc, mybir.MemoryLocationSet):
                continue
            name = alloc.memorylocations[0].name
            if alloc.kind == "ExternalInput":
                if name != part_name:
                    in_names.append(name)
            elif alloc.kind == "ExternalOutput":
                shape = tuple(alloc.tensor_shape)
                dtype = mybir.dt.np(alloc.dtype)
                out_names.append(name)
                out_avals.append(jax.core.ShapedArray(shape, dtype))
                zero_outs.append(np.zeros(shape, dtype))
        n_params = len(in_names)
        all_names = in_names + out_names
        if part_name is not None:
            all_names = all_names + [part_name]

        def _body(*args):
            operands = list(args)
            if part_name is not None:
                operands.append(bass2jax.partition_id_tensor())
            outs = bass2jax._bass_exec_p.bind(
                *operands, out_avals=tuple(out_avals),
                in_names=tuple(all_names), out_names=tuple(out_names),
                lowering_input_output_aliases=(),
                sim_require_finite=True, sim_require_nnan=True, nc=nc)
            return tuple(outs)

        devices = jax.devices()[:NCORES]
        mesh = Mesh(np.asarray(devices), ("core",))
        n_outs = len(out_names)
        sharded = jax.jit(
            shard_map(_body, mesh=mesh,
                      in_specs=(PartitionSpec("core"),) * (n_params + n_outs),
                      out_specs=(PartitionSpec("core"),) * n_outs,
                      check_rep=False),
            donate_argnums=tuple(range(n_params, n_params + n_outs)),
            keep_unused=True)
        _NC_CACHE["jit"][key] = (sharded, in_names, out_names, out_avals,
                                 zero_outs)

    sharded, in_names, out_names, out_avals, zero_outs = _NC_CACHE["jit"][key]
    concat_in = [np.concatenate([m[n] for m in in_maps], axis=0)
                 for n in in_names]
    concat_zeros = [np.zeros((NCORES * z.shape[0], *z.shape[1:]), z.dtype)
                    for z in zero_outs]
    out_arrs = sharded(*concat_in, *concat_zeros)
    return [
        {n: np.asarray(out_arrs[i]).reshape(NCORES, *out_avals[i].shape)[c]
         for i, n in enumerate(out_names)}
        for c in range(NCORES)
    ]


def _host_finish(osum, nyq, tail):
    """osum [B*3, 32, 64], nyq [B*3, T], tail [B*3, 128, 8] -> outputs."""
    inv = np.float32(1.0 / (PS * PS))
    tre = tail[:, :, 0::2]
    tim = tail[:, :, 1::2]
    tmag = np.sqrt(tre.astype(np.float64) ** 2 + tim.astype(np.float64) ** 2)
    mag_tail = np.empty((B * 3, 512), np.float64)
    for c in range(4):
        mag_tail[:, c::4] = tmag[:, :, c]
    mag_tail = np.maximum(mag_tail, EPS)
    nyq_mag = np.maximum(np.abs(nyq.astype(np.float64)), EPS)

    sums = np.zeros((B, 3, NPF, NPT), np.float64)
    sums[:, :, :32, :64] = osum.reshape(B, 3, 32, 64)
    for b in range(B):
        rows = {s: b * 3 + i for i, s in enumerate("stg")}
        for mi, (xa, xb, sqr) in enumerate(
                (("s", "g", False), ("t", "g", False), ("s", "t", True))):
            da = mag_tail[rows[xa]] - mag_tail[rows[xb]]
            dn = nyq_mag[rows[xa]] - nyq_mag[rows[xb]]
            va = da ** 2 if sqr else np.abs(da)
            vn = dn ** 2 if sqr else np.abs(dn)
            sums[b, mi, :32, 64] = va.reshape(32, 16).sum(axis=1)
            sums[b, mi, 32, :64] = vn[:TM].reshape(64, 16).sum(axis=1)
            sums[b, mi, 32, 64] = vn[TM]

    s2 = sums.reshape(B, 3, NPF * NPT).astype(np.float32)
    err_s = s2[:, 0] * inv
    err_t = s2[:, 1] * inv
    pl = s2[:, 2] * inv
    kgs = err_s - err_t
    order = np.argsort(-kgs, axis=1, kind="stable")[:, :KSEL]
    mask = np.zeros_like(kgs)
    np.put_along_axis(mask, order, 1.0, axis=1)
    selected = (pl * mask).sum(axis=1, dtype=np.float32)
    count = np.maximum(mask.sum(axis=1, dtype=np.float32), 1.0)
    loss = np.float32(np.mean(selected / count, dtype=np.float32))
    sel_ratio = np.float32(mask.mean(dtype=np.float32))
    kgs_mean = np.float32(kgs.mean(dtype=np.float32))
    kgs_pos_ratio = np.float32((kgs > 0).mean(dtype=np.float32))
    return loss, sel_ratio, kgs_mean, kgs_pos_ratio


def kernel(student_waveform, teacher_waveform, target_waveform,
           n_fft=1024, hop_length=256, patch_size=16):
    xs = np.ascontiguousarray(student_waveform, dtype=np.float32)
    xt = np.ascontiguousarray(teacher_waveform, dtype=np.float32)
    xg = np.ascontiguousarray(target_waveform, dtype=np.float32)

    nc = _get_nc()
    consts = _consts()
    in_maps = []
    for c in range(NCORES):
        m = {"xs": xs[RPC * c:RPC * (c + 1)],
             "xt": xt[RPC * c:RPC * (c + 1)],
             "xg": xg[RPC * c:RPC * (c + 1)]}
        m.update(consts)
        in_maps.append(m)

    results = _run_on_cores(nc, in_maps)

    osum = np.concatenate([r["osum"] for r in results], axis=0)
    nyq = np.concatenate([r["nyq"] for r in results], axis=0)
    tail = np.concatenate([r["tail"] for r in results], axis=0)
    return _host_finish(osum, nyq, tail)


# revision 3
# speedup vs baseline: 1.0469x; 1.0469x over previous
"""Trainium2 Bass kernel for the STFT patch-dispatch loss.

Same hop-folded DFT as the baseline, engineered around engine balance:

  - fp16 everywhere on the signal path (11-bit mantissa = f32r-grade
    precision at half the bytes): V, folds, DFT weights/operands, mags,
    patch diffs.  All big DVE ops run in 16-bit 2x mode.
  - vpp / vq computed with tensor_tensor_scan sliding-window recurrences
    straight from V (no vp intermediate).
  - Each DFT range lands re|im packed in one 2-bank PSUM tile -> one ACT
    Square per range + one fp16 DVE add -> |X|^2, then one ACT Sqrt per
    signal.
  - t in [0,1024) everywhere on device; the tail frame (t=1024) and the
    nyquist row (k=512) are shipped raw and folded in on the host.
  - Waveform loads consolidated into 5 DMAs per signal-row.

Sharding: batch rows 2c, 2c+1 -> core c (8 cores).
"""
import numpy as np

import concourse.bass as bass
import concourse.bacc as bacc
import concourse.mybir as mybir
from concourse import tile

dt = mybir.dt
Alu = mybir.AluOpType
Act = mybir.ActivationFunctionType

B, L = 16, 262144
NCORES = 8
RPC = B // NCORES
NFFT, HOP, PS = 1024, 256, 16
PAD = NFFT // 2
LP = L + 2 * PAD           # 263168
NBLK = LP // HOP           # 1028
T = 1 + (LP - NFFT) // HOP  # 1025 frames; main block = 1024, tail = 1
TM = T - 1                 # 1024
NPF, NPT = 33, 65
KSEL = max(1, int(NPF * NPT * 0.3))
EPS = 1e-08

WDT = dt.float16           # working dtype on the signal path
MAIN_RANGES = [(0, 512), (512, 1024)]


def _consts():
    r = np.arange(256)
    p = np.arange(128)
    wc = np.empty((256, 512), np.float32)
    ws = np.empty((256, 512), np.float32)
    for c in range(4):
        k = 4 * p + c
        ang = 2.0 * np.pi * np.outer(r, k) / NFFT
        wc[:, 128 * c:128 * (c + 1)] = np.cos(ang)
        ws[:, 128 * c:128 * (c + 1)] = -np.sin(ang)
    wc = wc.astype(np.float16)
    ws = ws.astype(np.float16)
    wcn = np.concatenate([-wc[:, 128:256], -wc[:, 384:512],
                          -wc[:, 256:384]], axis=1)
    wsn = np.concatenate([-ws[:, 128:256], -ws[:, 384:512],
                          -ws[:, 256:384]], axis=1)
    wn = np.where(r % 2 == 0, 1.0, -1.0).astype(np.float16).reshape(256, 1)
    ones4 = (p[:, None] // 4 == np.arange(32)[None, :]).astype(np.float32)
    ident = np.eye(128, dtype=np.float32)
    swap2 = np.array([[0.0, 1.0], [1.0, 0.0]], np.float32)
    return {
        "wc0": wc[:128], "wc1": wc[128:],
        "ws0": ws[:128], "ws1": ws[128:],
        "wn0": wn[:128], "wn1": wn[128:],
        "wcn0": wcn[:128], "wcn1": wcn[128:],
        "wsn0": wsn[:128], "wsn1": wsn[128:],
        "ones4": ones4, "ident": ident, "swap2": swap2,
    }


CONST_SPECS = {
    "wc0": [128, 512], "wc1": [128, 512],
    "ws0": [128, 512], "ws1": [128, 512],
    "wn0": [128, 1], "wn1": [128, 1],
    "wcn0": [128, 384], "wcn1": [128, 384],
    "wsn0": [128, 384], "wsn1": [128, 384],
    "ones4": [128, 32], "ident": [128, 128], "swap2": [2, 2],
}
FP16_CONSTS = ("wc0", "wc1", "ws0", "ws1", "wn0", "wn1",
               "wcn0", "wcn1", "wsn0", "wsn1")


def _seg(x_d, b, start, nrows):
    return x_d[b:b + 1, start:start + 256 * nrows].rearrange(
        "o (m r) -> (o m) r", r=256)


def build_nc(repeat=1):
    nc = bacc.Bacc("TRN2", target_bir_lowering=False, debug=False,
                   num_devices=NCORES)

    x_d = {s: nc.dram_tensor(f"x{s}", [RPC, L], dt.float32,
                             kind="ExternalInput") for s in "stg"}
    c_d = {n: nc.dram_tensor(
        n, shp, dt.float16 if n in FP16_CONSTS else dt.float32,
        kind="ExternalInput") for n, shp in CONST_SPECS.items()}
    osum_d = nc.dram_tensor("osum", [RPC * 3, 32, 64], dt.float32,
                            kind="ExternalOutput")

    with tile.TileContext(nc) as tc:
        with (
            tc.tile_pool(name="const", bufs=1) as cp,
            tc.tile_pool(name="vwork", bufs=2) as vp_,
            tc.tile_pool(name="fwork", bufs=2) as fp_,
            tc.tile_pool(name="upool", bufs=3) as up,
            tc.tile_pool(name="magpool", bufs=2) as magp,
            tc.tile_pool(name="sqpool", bufs=3) as sqp,
            tc.tile_pool(name="dpool", bufs=3) as dp_,
            tc.tile_pool(name="redpool", bufs=3) as redp,
            tc.tile_pool(name="dft_ps", bufs=2, space="PSUM") as dft_ps,
            tc.tile_pool(name="tr_ps", bufs=2, space="PSUM") as tr_ps,
            tc.tile_pool(name="pa_ps", bufs=2, space="PSUM") as pa_ps,
        ):
            C = {}
            for n, shp in CONST_SPECS.items():
                cdt = dt.float16 if n in FP16_CONSTS else dt.float32
                C[n] = cp.tile(shp, cdt, tag=n, name=f"c_{n}")
                nc.sync.dma_start(C[n][:], c_d[n][:])

            def rev2(seg_start, s, b):
                """[2,256]: rows = rev(seg rows) in place; the consumer
                transposes with `swap2` so the row order comes out right."""
                sc = up.tile([2, 256], dt.float32, tag="sc", name="sc")
                nc.sync.dma_start(sc[:], _seg(x_d[s], b, seg_start, 2))
                ur = up.tile([2, 256], dt.float32, tag="ur", name="ur")
                nc.vector.tensor_copy(ur[:], sc[0:2, 255::-1])
                return ur

            def build_V(s, b):
                """V[r, h, m]: [128, 2, NBLK] fp16 tile."""
                v = vp_.tile([128, 2, NBLK], WDT, tag="v")
                uh = rev2(1, s, b)
                u0 = up.tile([128, 256], dt.float32, tag="u0", name="u0")
                nc.sync.dma_start(u0[0:126, :], _seg(x_d[s], b, 0, 126))
                um = up.tile([128, 7, 256], dt.float32, tag="um", name="um")
                nc.sync.dma_start(
                    um[:], x_d[s][b:b + 1, 32256:32256 + 7 * 32768].rearrange(
                        "o (g m r) -> (o m) g r", m=128, r=256))
                ut = up.tile([2, 256], dt.float32, tag="ut", name="ut")
                nc.sync.dma_start(ut[:], _seg(x_d[s], b, 261632, 2))
                ub = rev2(261631, s, b)
                groups = [
                    (0, [(uh[0:2, :], 2, True), (u0[0:126, :], 126, False),
                         (um[:, 0, :], 128, False), (um[:, 1, :], 128, False),
                         (um[:, 2, :], 128, False)]),
                    (512, [(um[:, 3, :], 128, False), (um[:, 4, :], 128, False),
                           (um[:, 5, :], 128, False),
                           (um[:, 6, :], 128, False)]),
                    (1024, [(ut[0:2, :], 2, False), (ub[0:2, :], 2, True)]),
                ]
                for col, pieces in groups:
                    width = sum(nr for _, nr, _ in pieces)
                    for h in (0, 1):
                        tp = tr_ps.tile([128, 512], dt.float32, tag="trp",
                                        name="tp")
                        off = 0
                        for uap, nr, swp in pieces:
                            rhs = (C["swap2"][0:2, 0:2] if swp
                                   else C["ident"][0:nr, 0:nr])
                            nc.tensor.transpose(
                                tp[:, off:off + nr],
                                uap[:, 128 * h:128 * h + 128], rhs)
                            off += nr
                        nc.scalar.copy(v[:, h, col:col + width],
                                       tp[:, 0:width])
                return v

            def fold_ops(v):
                """vm [128,2,1026] (TT); vpp, vq [128,2,1025] (scans)."""
                vm = fp_.tile([128, 2, NBLK - 2], WDT, tag="vm")
                nc.vector.tensor_sub(vm[:], v[:, :, 0:NBLK - 2],
                                     v[:, :, 2:NBLK])
                vpp = fp_.tile([128, 2, T], WDT, tag="vpp")
                vq = fp_.tile([128, 2, T], WDT, tag="vq")
                a2 = fp_.tile([128, 2, 2], WDT, tag="a2")
                nc.vector.tensor_add(a2[:], v[:, :, 0:2], v[:, :, 2:4])
                nc.vector.tensor_add(vpp[:, :, 0:1], a2[:, :, 0:1],
                                     a2[:, :, 1:2])
                nc.vector.tensor_sub(vq[:, :, 0:1], a2[:, :, 0:1],
                                     a2[:, :, 1:2])
                for h in (0, 1):
                    # vpp[t] = (v[t+3] + vpp[t-1]) - v[t-1]
                    nc.vector.tensor_tensor_scan(
                        vpp[:, h, 1:T], v[:, h, 4:3 + T], v[:, h, 0:T - 1],
                        initial=vpp[:, h, 0:1], op0=Alu.add,
                        op1=Alu.subtract)
                    # vq[t] = (v[t-1] - vq[t-1]) - v[t+3]
                    nc.vector.tensor_tensor_scan(
                        vq[:, h, 1:T], v[:, h, 0:T - 1], v[:, h, 4:3 + T],
                        initial=vq[:, h, 0:1], op0=Alu.subtract,
                        op1=Alu.subtract)
                return vm, vpp, vq

            def chunk_terms(c):
                cs = slice(128 * c, 128 * (c + 1))
                if c % 2 == 0:
                    return ([("wc", cs, 0)], [("ws", cs, 0)])
                ns = slice(0, 128) if c == 1 else slice(128, 256)
                if c == 1:
                    return ([("wc", cs, 0), ("ws", cs, 1)],
                            [("ws", cs, 0), ("wcn", ns, 1)])
                return ([("wc", cs, 0), ("wsn", ns, 1)],
                        [("ws", cs, 0), ("wc", cs, 1)])

            def chunk_mag(ops, c, mag2, score, sig_idx):
                """DFT chunk c -> mag2[:, c*1024 + t] (fp16) + tail cols."""
                vm, vpp, vq = ops
                terms_re, terms_im = chunk_terms(c)
                src = (vpp if c == 0 else vq) if c % 2 == 0 else vm
                for ri, (lo, hi) in enumerate(MAIN_RANGES):
                    ps = dft_ps.tile([128, 1024], dt.float32, tag="dftp",
                                     name="psx")
                    for xi, terms in enumerate((terms_re, terms_im)):
                        nmm = 2 * len(terms)
                        k = 0
                        for wname, wsl, shift in terms:
                            for h in (0, 1):
                                nc.tensor.matmul(
                                    ps[:, 512 * xi:512 * xi + 512],
                                    C[wname + str(h)][:, wsl],
                                    src[:, h, lo + shift:hi + shift],
                                    start=(k == 0), stop=(k == nmm - 1))
                                k += 1
                    sq = sqp.tile([128, 1024], WDT, tag="sqt", name="sq")
                    nc.scalar.activation(sq[:], ps[:], Act.Square)
                    nc.vector.tensor_add(
                        mag2[:, c * TM + lo:c * TM + hi],
                        sq[:, 0:512], sq[:, 512:1024])
            def signal_mags(ops, sig_idx, b, score):
                """All 4 chunks -> mag [128, 4096] fp16 (+ tail, nyq out)."""
                mag2 = magp.tile([128, 4 * TM], WDT, tag="mag2",
                                 name="mag2")
                for c in range(4):
                    chunk_mag(ops, c, mag2, score, sig_idx)
                mag = magp.tile([128, 4 * TM], WDT, tag=f"mag{sig_idx}",
                                name=f"mag{sig_idx}")
                nc.scalar.activation(mag[:], mag2[:], Act.Sqrt)
                return mag

            def patch(b, ms, mt, mg, score):
                for mi, (ta, tb, sqr) in enumerate(
                        ((ms, mg, False), (mt, mg, False), (ms, mt, True))):
                    d = dp_.tile([128, 4 * TM], WDT, tag="d", name=f"d{mi}")
                    nc.vector.tensor_sub(d[:], ta[:], tb[:])
                    if sqr:
                        d2 = dp_.tile([128, 4 * TM], WDT, tag="d",
                                      name="d2")
                        nc.vector.tensor_mul(d2[:], d[:], d[:])
                        d = d2
                    red = redp.tile([128, 256], dt.float32, tag="red",
                                    name=f"red{mi}")
                    nc.vector.tensor_reduce(
                        red[:],
                        d[:].rearrange("p (a t) -> p a t", t=16),
                        axis=mybir.AxisListType.X, op=Alu.add,
                        apply_absolute_value=not sqr)
                    pps = score[0:32, 64 * mi:64 * mi + 64]
                    for c in range(4):
                        nc.tensor.matmul(pps, C["ones4"][:],
                                         red[:, 64 * c:64 * (c + 1)],
                                         start=(c == 0), stop=(c == 3))
                    idx = b * 3 + mi
                    osb = redp.tile([32, 64], dt.float32, tag="osb",
                                    name="osb")
                    nc.vector.tensor_copy(osb[:], pps)
                    nc.sync.dma_start(
                        osum_d[idx:idx + 1].rearrange("o p f -> (o p) f"),
                        osb[:])

            def row_stage_a(b):
                score = pa_ps.tile([128, 288], dt.float32, tag="score",
                                   name="score")
                vs = build_V("s", b)
                ops = fold_ops(vs)
                vt = build_V("t", b)
                ms = signal_mags(ops, 0, b, score)
                ops = fold_ops(vt)
                vg = build_V("g", b)
                mt_ = signal_mags(ops, 1, b, score)
                ops = fold_ops(vg)
                mg = signal_mags(ops, 2, b, score)
                return (b, ms, mt_, mg, score)

            def body():
                for b in range(RPC):
                    patch(*row_stage_a(b))

            if repeat == 1:
                body()
            else:
                with tc.For_i(0, repeat, 1):
                    body()

    nc.compile()
    return nc


_NC_CACHE = {}


def _get_nc():
    if "nc" not in _NC_CACHE:
        _NC_CACHE["nc"] = build_nc()
    return _NC_CACHE["nc"]


def _run_on_cores(nc, in_maps):
    """Execute via cached PJRT callable (axon) with jit reuse."""
    from concourse.bass_utils import axon_active

    if not axon_active():
        from concourse.bass_utils import run_bass_kernel_spmd
        return run_bass_kernel_spmd(nc, in_maps,
                                    core_ids=list(range(NCORES))).results

    import jax
    from jax.sharding import Mesh, PartitionSpec
    from jax.experimental.shard_map import shard_map
    from concourse import bass2jax

    key = id(nc)
    if key not in _NC_CACHE.setdefault("jit", {}):
        bass2jax.install_neuronx_cc_hook()
        part_name = (nc.partition_id_tensor.name
                     if nc.partition_id_tensor else None)
        in_names, out_names, out_avals, zero_outs = [], [], [], []
        for alloc in nc.m.functions[0].allocations:
            if not isinstance(alloc, mybir.MemoryLocationSet):
                continue
            name = alloc.memorylocations[0].name
            if alloc.kind == "ExternalInput":
                if name != part_name:
                    in_names.append(name)
            elif alloc.kind == "ExternalOutput":
                shape = tuple(alloc.tensor_shape)
                dtype = mybir.dt.np(alloc.dtype)
                out_names.append(name)
                out_avals.append(jax.core.ShapedArray(shape, dtype))
                zero_outs.append(np.zeros(shape, dtype))
        n_params = len(in_names)
        all_names = in_names + out_names
        if part_name is not None:
            all_names = all_names + [part_name]

        def _body(*args):
            operands = list(args)
            if part_name is not None:
                operands.append(bass2jax.partition_id_tensor())
            outs = bass2jax._bass_exec_p.bind(
                *operands, out_avals=tuple(out_avals),
                in_names=tuple(all_names), out_names=tuple(out_names),
                lowering_input_output_aliases=(),
                sim_require_finite=True, sim_require_nnan=True, nc=nc)
            return tuple(outs)

        devices = jax.devices()[:NCORES]
        mesh = Mesh(np.asarray(devices), ("core",))
        n_outs = len(out_names)
        sharded = jax.jit(
            shard_map(_body, mesh=mesh,
                      in_specs=(PartitionSpec("core"),) * (n_params + n_outs),
                      out_specs=(PartitionSpec("core"),) * n_outs,
                      check_rep=False),
            donate_argnums=tuple(range(n_params, n_params + n_outs)),
            keep_unused=True)
        _NC_CACHE["jit"][key] = (sharded, in_names, out_names, out_avals,
                                 zero_outs)

    sharded, in_names, out_names, out_avals, zero_outs = _NC_CACHE["jit"][key]
    concat_in = [np.concatenate([m[n] for m in in_maps], axis=0)
                 for n in in_names]
    concat_zeros = [np.zeros((NCORES * z.shape[0], *z.shape[1:]), z.dtype)
                    for z in zero_outs]
    out_arrs = sharded(*concat_in, *concat_zeros)
    return [
        {n: np.asarray(out_arrs[i]).reshape(NCORES, *out_avals[i].shape)[c]
         for i, n in enumerate(out_names)}
        for c in range(NCORES)
    ]


def _host_edges(x):
    """Edge spectra computed on host from the raw waveform x [B, L]:
    (mag_tail [B, 513] = |X[:, 1024]|, nyq_mag [B, T] = |X[512, :]|).
    The tail is one rfft per row; the nyquist row is an alternating-sign
    sliding-window sum (hop 256 keeps the global parity aligned)."""
    xp = np.pad(x.astype(np.float64), ((0, 0), (PAD, PAD)), mode="reflect")
    tail = np.abs(np.fft.rfft(xp[:, TM * HOP:TM * HOP + NFFT], axis=-1))
    alt = xp * np.where(np.arange(LP) % 2 == 0, 1.0, -1.0)[None, :]
    S = np.cumsum(alt, axis=-1)
    idx_hi = np.arange(T) * HOP + NFFT - 1
    nyq = S[:, idx_hi].copy()
    nz = np.arange(1, T) * HOP - 1
    nyq[:, 1:] -= S[:, nz]
    return np.maximum(tail, EPS), np.maximum(np.abs(nyq), EPS)


def _host_finish(osum, xs, xt, xg):
    """osum [B*3, 32, 64] from device + host-side edge columns."""
    inv = np.float32(1.0 / (PS * PS))
    edges = {s: _host_edges(x) for s, x in (("s", xs), ("t", xt), ("g", xg))}

    sums = np.zeros((B, 3, NPF, NPT), np.float64)
    sums[:, :, :32, :64] = osum.reshape(B, 3, 32, 64)
    for b in range(B):
        for mi, (xa, xb, sqr) in enumerate(
                (("s", "g", False), ("t", "g", False), ("s", "t", True))):
            ta_, na = edges[xa][0][b], edges[xa][1][b]
            tb_, nb = edges[xb][0][b], edges[xb][1][b]
            da = ta_[:512] - tb_[:512]
            dn = na - nb
            va = da ** 2 if sqr else np.abs(da)
            vn = dn ** 2 if sqr else np.abs(dn)
            sums[b, mi, :32, 64] = va.reshape(32, 16).sum(axis=1)
            sums[b, mi, 32, :64] = vn[:TM].reshape(64, 16).sum(axis=1)
            sums[b, mi, 32, 64] = abs(ta_[512] - tb_[512]) ** (
                2 if sqr else 1)

    s2 = sums.reshape(B, 3, NPF * NPT).astype(np.float32)
    err_s = s2[:, 0] * inv
    err_t = s2[:, 1] * inv
    pl = s2[:, 2] * inv
    kgs = err_s - err_t
    order = np.argsort(-kgs, axis=1, kind="stable")[:, :KSEL]
    mask = np.zeros_like(kgs)
    np.put_along_axis(mask, order, 1.0, axis=1)
    selected = (pl * mask).sum(axis=1, dtype=np.float32)
    count = np.maximum(mask.sum(axis=1, dtype=np.float32), 1.0)
    loss = np.float32(np.mean(selected / count, dtype=np.float32))
    sel_ratio = np.float32(mask.mean(dtype=np.float32))
    kgs_mean = np.float32(kgs.mean(dtype=np.float32))
    kgs_pos_ratio = np.float32((kgs > 0).mean(dtype=np.float32))
    return loss, sel_ratio, kgs_mean, kgs_pos_ratio


def kernel(student_waveform, teacher_waveform, target_waveform,
           n_fft=1024, hop_length=256, patch_size=16):
    xs = np.ascontiguousarray(student_waveform, dtype=np.float32)
    xt = np.ascontiguousarray(teacher_waveform, dtype=np.float32)
    xg = np.ascontiguousarray(target_waveform, dtype=np.float32)

    nc = _get_nc()
    consts = _consts()
    in_maps = []
    for c in range(NCORES):
        m = {"xs": xs[RPC * c:RPC * (c + 1)],
             "xt": xt[RPC * c:RPC * (c + 1)],
             "xg": xg[RPC * c:RPC * (c + 1)]}
        m.update(consts)
        in_maps.append(m)

    results = _run_on_cores(nc, in_maps)

    osum = np.concatenate([r["osum"] for r in results], axis=0)
    return _host_finish(osum, xs, xt, xg)
